# revision 10
# baseline (speedup 1.0000x reference)
"""Linformer attention Bass/Trainium2 kernel (8 NeuronCores, SPMD).

Math per (b, h):
  Kp = E[h] @ (K[b,h] * m[b])            [LK, D]     (contraction over S)
  Vp = E[h] @ (V[b,h] * m[b])            [LK, D]
  dot = Q[b,h] @ Kp.T / sqrt(D)          [S, LK]
  attn = softmax(dot, axis=-1)
  out = attn @ Vp                        [S, D]

Shapes: B=2, H=12, S=4096, D=64, LK=256.

Sharding: 24 (h,b) pairs -> 8 cores x 3 pairs, h-major so each core touches
exactly 2 distinct heads (pairs ordered [dupA, dupA, single] so the SPMD
program reuses the first E head for pairs 0,1 and loads the second for pair 2).

Device-side layouts (host prepacks; s is tiled s = c*128 + p):
  ET  [2, 128, 32*256] : ET[i][p, c*256+k]  = E[head_i, k, c*128+p]   (E^T)
  KV  [3, 128, 32*128] : KV[pr][p, c*128+j] = concat(K,V)[b,h,c*128+p,j]
  QTP [3, 128, 4*512]  : QTP[pr][64*half+d, s'] = Q[b,h, 2048*half+s', d]
  MK  [3, 128, 32]     : MK[pr][p, c]       = mask[b, c*128+p]
  O   [3, 128, 32*64]  : O[pr][p, c*64+d]   = out[b,h, c*128+p, d]

On-device pipeline per pair:
  1. kv *= mask (per-partition broadcast)
  2. proj: psum[128,256] += kv_chunk[128s,128]^T @ et_chunk[128s,256]
     -> rows 0:64 = Kp^T[d,k], rows 64:128 = Vp^T[d,k]
  3. Vp^T -> PE-transpose -> Vp[k,d]; augment with ones column (rowsum trick)
  4. per s-chunk 512: dot^T[k_half,s] = Kp^T_half.T @ Q^T_chunk (2 matmuls),
     exp on ACT (scale=1/8, no max-subtraction: |dot/8| < ~10, fp32-safe),
     out_un^T[65,s] = [Vp;1]^T-augmented AV matmul over both k halves,
     PE-transpose 128-col pieces -> [128, 65], normalize by 1/rowsum
     (per-partition scalar) -> natural [s, d] layout, stage to osb.
  5. one 1MB DMA of osb -> O[pr]
"""

import sys

if "/opt/trn_rl_repo" not in sys.path:
    sys.path.insert(0, "/opt/trn_rl_repo")

import numpy as np

import concourse.bass as bass
import concourse.bacc as bacc
import concourse.mybir as mybir
import concourse.tile as tile
from concourse.bass_utils import run_bass_kernel_spmd
from concourse.masks import make_identity

FP = mybir.dt.float32

B, H, S, D, LK = 2, 12, 4096, 64, 256
NC = 8
PAIRS = 3          # (b,h) pairs per core
P = 128            # partitions
SC = S // P        # 32 s-chunks of 128
NSC = S // 512     # 8 s-chunks of 512

_nc_cache = None


def build_nc():
    nc = bacc.Bacc(None, target_bir_lowering=False, debug=False)
    ET = nc.declare_dram_parameter("ET", [2, P, SC * LK], FP, isOutput=False)
    KV = nc.declare_dram_parameter("KV", [PAIRS, P, SC * P], FP, isOutput=False)
    QTP = nc.declare_dram_parameter("QTP", [PAIRS, P, 4 * 512], FP, isOutput=False)
    O = nc.declare_dram_parameter("O", [PAIRS, D, S], FP, isOutput=True)

    with tile.TileContext(nc) as tc:
        with (
            tc.tile_pool(name="etp", bufs=2) as etp,
            tc.tile_pool(name="big", bufs=2) as big,
            tc.tile_pool(name="small", bufs=2) as small,
            tc.tile_pool(name="hot", bufs=6) as hot,
            tc.tile_pool(name="outp", bufs=3) as outp,
            tc.tile_pool(name="singles", bufs=1) as singles,
            tc.tile_pool(name="psd", bufs=4, space="PSUM") as psd,
            tc.tile_pool(name="psa", bufs=2, space="PSUM") as psa,
            tc.tile_pool(name="psb", bufs=2, space="PSUM") as psb,
        ):
            ident = singles.tile([P, P], FP)
            make_identity(nc, ident)
            ones_sb = singles.tile([1, D], FP)
            nc.vector.memset(ones_sb[:], 1.0)

            et_tiles = [
                etp.tile([P, SC * LK], FP, tag="et", name=f"et{i}") for i in range(2)
            ]
            nc.sync.dma_start(et_tiles[0][:], ET[0])

            for pr in range(PAIRS):
                if pr == 1:
                    nc.sync.dma_start(et_tiles[1][:], ET[1])
                et_sb = et_tiles[0 if pr < 2 else 1]

                kv_sb = big.tile([P, SC * P], FP, tag="kv")
                nc.sync.dma_start(kv_sb[:], KV[pr])
                qtp_sb = big.tile([P, 4 * 512], FP, tag="qtp")
                nc.sync.dma_start(qtp_sb[:], QTP[pr])

                # projection: accumulate over all 32 s-chunks
                ps_p = psa.tile([P, LK], FP, tag="av")
                for c in range(SC):
                    nc.tensor.matmul(
                        ps_p[:],
                        kv_sb[:, c * P : (c + 1) * P],
                        et_sb[:, c * LK : (c + 1) * LK],
                        start=(c == 0),
                        stop=(c == SC - 1),
                    )

                # proj_sb rows 0:64 = Kp^T, rows 64:128 = Vp^T
                proj_sb = small.tile([P, LK], FP, tag="proj")
                nc.vector.tensor_copy(proj_sb[:], ps_p[:])
                # duplicate Kp^T into partitions 64:128 for the high s-half matmuls
                kpt_hi = small.tile([P, LK], FP, tag="kpt_hi")
                nc.sync.dma_start(kpt_hi[64:128, :], proj_sb[0:64, :])

                # Vp^T [64,256] -> Vp [256,64] as two [128,65] lhsT tiles
                # (column 64 = ones row for the softmax denominator)
                vpa = []
                for kh in range(2):
                    ps_t = psb.tile([P, 64], FP, tag="bc")
                    nc.tensor.transpose(
                        ps_t[:],
                        proj_sb[64:128, kh * P : (kh + 1) * P],
                        ident[64:128, 64:128],
                    )
                    va = small.tile([P, 65], FP, tag=f"vpa{kh}")
                    nc.vector.tensor_copy(va[:, 0:64], ps_t[:])
                    nc.vector.memset(va[:, 64:65], 1.0)
                    vpa.append(va)

                osb = outp.tile([D, S], FP, tag="osb")

                # 8 s-chunks of 512, in groups of 2 sharing dot weights;
                # groups alternate low/high s-halves so consecutive dot
                # matmuls use disjoint PE row groups (partitions 0:64/64:128)
                for half, locs in ((0, (0, 1)), (1, (0, 1)), (0, (2, 3)), (1, (2, 3))):
                    pb = 64 * half
                    lhs_src = kpt_hi if half else proj_sb
                    ps_d = {}
                    for kh in range(2):
                        for loc in locs:
                            pd = psd.tile([P, 512], FP, tag="dot", name=f"pd{kh}{loc}")
                            nc.tensor.matmul(
                                pd[:],
                                lhs_src[pb : pb + 64, kh * P : (kh + 1) * P],
                                qtp_sb[pb : pb + 64, loc * 512 : (loc + 1) * 512],
                                start=True,
                                stop=True,
                            )
                            ps_d[kh, loc] = pd
                    exps = {}
                    for kh in range(2):
                        for loc in locs:
                            ex = hot.tile([P, 512], FP, tag="exp", name=f"ex{kh}{loc}")
                            nc.scalar.activation(
                                ex[:],
                                ps_d[kh, loc][:],
                                mybir.ActivationFunctionType.Exp,
                                scale=0.125,
                            )
                            exps[kh, loc] = ex
                    ps_o = {
                        loc: psa.tile([65, 512], FP, tag="av", name=f"po{loc}")
                        for loc in locs
                    }
                    for kh in range(2):
                        for loc in locs:
                            nc.tensor.matmul(
                                ps_o[loc][:],
                                vpa[kh][:],
                                exps[kh, loc][:],
                                start=(kh == 0),
                                stop=(kh == 1),
                            )
                    for loc in locs:
                        s0 = half * 2048 + loc * 512
                        rrow = hot.tile([1, 512], FP, tag="rrow")
                        nc.vector.reciprocal(rrow[:], ps_o[loc][64:65, :])
                        bc = psb.tile([D, 512], FP, tag="bc")
                        nc.tensor.matmul(bc[:], ones_sb[:], rrow[:], start=True, stop=True)
                        bc_sb = hot.tile([D, 512], FP, tag="bcs")
                        nc.scalar.copy(bc_sb[:], bc[:])
                        nc.vector.tensor_mul(
                            osb[:, s0 : s0 + 512],
                            ps_o[loc][0:64, :],
                            bc_sb[:],
                        )
                nc.sync.dma_start(O[pr], osb[:])

    return nc


def _get_nc():
    global _nc_cache
    if _nc_cache is None:
        _nc_cache = build_nc()
        _nc_cache.compile()
    return _nc_cache


def _order_flats(flats):
    """Order so the duplicated head's two pairs come first."""
    hs = [f // 2 for f in flats]
    dup = next(h for h in hs if hs.count(h) == 2)
    return [f for f in flats if f // 2 == dup] + [f for f in flats if f // 2 != dup]


def _pack_s(x):
    """[S, F] -> [128, 32*F] with row p, col c*F+f = x[c*128+p, f]."""
    Sdim, F = x.shape
    return np.ascontiguousarray(
        x.reshape(SC, P, F).transpose(1, 0, 2).reshape(P, SC * F)
    )


def make_in_maps(Q, K, V, mask, E):
    Q = np.asarray(Q, np.float32)
    K = np.asarray(K, np.float32)
    V = np.asarray(V, np.float32)
    mask = np.asarray(mask, np.float32)
    E = np.asarray(E, np.float32)
    in_maps, metas = [], []
    for c in range(NC):
        flats = _order_flats([3 * c, 3 * c + 1, 3 * c + 2])
        pairs = [divmod(f, 2) for f in flats]  # (h, b)
        heads = [flats[0] // 2, flats[2] // 2]
        ET = np.stack([_pack_s(np.ascontiguousarray(E[h].T)) for h in heads])
        KVm = np.stack(
            [
                _pack_s(
                    np.concatenate([K[b, h], V[b, h]], axis=-1)
                    * mask[b][:, None]
                )
                for h, b in pairs
            ]
        )
        QTP = np.stack(
            [
                np.concatenate([Q[b, h].T[:, :2048], Q[b, h].T[:, 2048:]], axis=0)
                for h, b in pairs
            ]
        )
        in_maps.append(
            {
                "ET": np.ascontiguousarray(ET),
                "KV": np.ascontiguousarray(KVm),
                "QTP": np.ascontiguousarray(QTP),
            }
        )
        metas.append(pairs)
    return in_maps, metas


def unshard(results, metas):
    out = np.empty((B, H, S, D), np.float32)
    for c in range(NC):
        for i, (h, b) in enumerate(metas[c]):
            out[b, h] = results[c]["O"][i].T
    return out


def kernel(Q, K, V, mask, E, **run_kwargs):
    nc = _get_nc()
    in_maps, metas = make_in_maps(Q, K, V, mask, E)
    res = run_bass_kernel_spmd(nc, in_maps, core_ids=list(range(NC)), **run_kwargs)
    out = unshard(res.results, metas)
    kernel.last_result = res
    return out


# revision 11
# speedup vs baseline: 1.0191x; 1.0191x over previous
"""Linformer attention Bass/Trainium2 kernel (8 NeuronCores, SPMD).

Math per (b, h):
  Kp = E[h] @ (K[b,h] * m[b])            [LK, D]     (contraction over S)
  Vp = E[h] @ (V[b,h] * m[b])            [LK, D]
  dot = Q[b,h] @ Kp.T / sqrt(D)          [S, LK]
  attn = softmax(dot, axis=-1)
  out = attn @ Vp                        [S, D]

Shapes: B=2, H=12, S=4096, D=64, LK=256.

Sharding: 24 (h,b) pairs -> 8 cores x 3 pairs, h-major so each core touches
exactly 2 distinct heads (pairs ordered [dupA, dupA, single] so the SPMD
program reuses the first E head for pairs 0,1 and loads the second for pair 2).

Device-side layouts (host prepacks; s is tiled s = c*128 + p):
  ET  [2, 128, 32*256] : ET[i][p, c*256+k]  = E[head_i, k, c*128+p]   (E^T)
  KV  [3, 128, 32*128] : KV[pr][p, c*128+j] = concat(K,V)[b,h,c*128+p,j]
  QTP [3, 128, 4*512]  : QTP[pr][64*half+d, s'] = Q[b,h, 2048*half+s', d]
  MK  [3, 128, 32]     : MK[pr][p, c]       = mask[b, c*128+p]
  O   [3, 128, 32*64]  : O[pr][p, c*64+d]   = out[b,h, c*128+p, d]

On-device pipeline per pair:
  1. kv *= mask (per-partition broadcast)
  2. proj: psum[128,256] += kv_chunk[128s,128]^T @ et_chunk[128s,256]
     -> rows 0:64 = Kp^T[d,k], rows 64:128 = Vp^T[d,k]
  3. Vp^T -> PE-transpose -> Vp[k,d]; augment with ones column (rowsum trick)
  4. per s-chunk 512: dot^T[k_half,s] = Kp^T_half.T @ Q^T_chunk (2 matmuls),
     exp on ACT (scale=1/8, no max-subtraction: |dot/8| < ~10, fp32-safe),
     out_un^T[65,s] = [Vp;1]^T-augmented AV matmul over both k halves,
     PE-transpose 128-col pieces -> [128, 65], normalize by 1/rowsum
     (per-partition scalar) -> natural [s, d] layout, stage to osb.
  5. one 1MB DMA of osb -> O[pr]
"""

import sys

if "/opt/trn_rl_repo" not in sys.path:
    sys.path.insert(0, "/opt/trn_rl_repo")

import numpy as np

import concourse.bass as bass
import concourse.bacc as bacc
import concourse.mybir as mybir
import concourse.tile as tile
from concourse.bass_utils import run_bass_kernel_spmd
from concourse.masks import make_identity

FP = mybir.dt.float32

B, H, S, D, LK = 2, 12, 4096, 64, 256
NC = 8
PAIRS = 3          # (b,h) pairs per core
P = 128            # partitions
SC = S // P        # 32 s-chunks of 128
NSC = S // 512     # 8 s-chunks of 512

_nc_cache = None


def build_nc():
    nc = bacc.Bacc(None, target_bir_lowering=False, debug=False)
    ET = nc.declare_dram_parameter("ET", [2, P, SC * LK], FP, isOutput=False)
    KV = nc.declare_dram_parameter("KV", [PAIRS, P, SC * P], FP, isOutput=False)
    QTP = nc.declare_dram_parameter("QTP", [PAIRS, P, 4 * 512], FP, isOutput=False)
    O = nc.declare_dram_parameter("O", [PAIRS, D, S], FP, isOutput=True)

    with tile.TileContext(nc) as tc:
        with (
            tc.tile_pool(name="etp", bufs=2) as etp,
            tc.tile_pool(name="big", bufs=2) as big,
            tc.tile_pool(name="small", bufs=2) as small,
            tc.tile_pool(name="hot", bufs=6) as hot,
            tc.tile_pool(name="outp", bufs=3) as outp,
            tc.tile_pool(name="singles", bufs=1) as singles,
            tc.tile_pool(name="psd", bufs=4, space="PSUM") as psd,
            tc.tile_pool(name="psa", bufs=2, space="PSUM") as psa,
            tc.tile_pool(name="psb", bufs=2, space="PSUM") as psb,
        ):
            ident = singles.tile([P, P], FP)
            make_identity(nc, ident)
            ones_sb = singles.tile([1, D], FP)
            nc.vector.memset(ones_sb[:], 1.0)

            et_tiles = [
                etp.tile([P, SC * LK], FP, tag="et", name=f"et{i}") for i in range(2)
            ]
            nc.sync.dma_start(et_tiles[0][:], ET[0])

            for pr in range(PAIRS):
                if pr == 1:
                    nc.sync.dma_start(et_tiles[1][:], ET[1])
                et_sb = et_tiles[0 if pr < 2 else 1]

                kv_sb = big.tile([P, SC * P], FP, tag="kv")
                nc.sync.dma_start(kv_sb[:], KV[pr])
                qtp_sb = big.tile([P, 4 * 512], FP, tag="qtp")
                nc.sync.dma_start(qtp_sb[:], QTP[pr])

                # projection: accumulate over all 32 s-chunks
                ps_p = psa.tile([P, LK], FP, tag="av")
                for c in range(SC):
                    nc.tensor.matmul(
                        ps_p[:],
                        kv_sb[:, c * P : (c + 1) * P],
                        et_sb[:, c * LK : (c + 1) * LK],
                        start=(c == 0),
                        stop=(c == SC - 1),
                    )

                # proj_sb rows 0:64 = Kp^T, rows 64:128 = Vp^T
                proj_sb = small.tile([P, LK], FP, tag="proj")
                nc.vector.tensor_copy(proj_sb[:], ps_p[:])
                # duplicate Kp^T into partitions 64:128 for the high s-half matmuls
                kpt_hi = small.tile([P, LK], FP, tag="kpt_hi")
                nc.sync.dma_start(kpt_hi[64:128, :], proj_sb[0:64, :])

                # Vp^T [64,256] -> Vp [256,64] as two [128,65] lhsT tiles
                # (column 64 = ones row for the softmax denominator)
                vpa = []
                for kh in range(2):
                    ps_t = psb.tile([P, 64], FP, tag="bc")
                    nc.tensor.transpose(
                        ps_t[:],
                        proj_sb[64:128, kh * P : (kh + 1) * P],
                        ident[64:128, 64:128],
                    )
                    va = small.tile([P, 65], FP, tag=f"vpa{kh}")
                    nc.vector.tensor_copy(va[:, 0:64], ps_t[:])
                    nc.vector.memset(va[:, 64:65], 1.0)
                    vpa.append(va)

                osb = outp.tile([D, S], FP, tag="osb")

                # 8 s-chunks of 512, in groups of 2 sharing dot weights;
                # groups alternate low/high s-halves so consecutive dot
                # matmuls use disjoint PE row groups (partitions 0:64/64:128)
                for half, locs in ((0, (0, 1)), (1, (0, 1)), (0, (2, 3)), (1, (2, 3))):
                    pb = 64 * half
                    lhs_src = kpt_hi if half else proj_sb
                    ps_d = {}
                    for kh in range(2):
                        for loc in locs:
                            pd = psd.tile([P, 512], FP, tag="dot", name=f"pd{kh}{loc}")
                            nc.tensor.matmul(
                                pd[:],
                                lhs_src[pb : pb + 64, kh * P : (kh + 1) * P],
                                qtp_sb[pb : pb + 64, loc * 512 : (loc + 1) * 512],
                                start=True,
                                stop=True,
                            )
                            ps_d[kh, loc] = pd
                    exps = {}
                    for kh in range(2):
                        for loc in locs:
                            ex = hot.tile([P, 512], FP, tag="exp", name=f"ex{kh}{loc}")
                            nc.scalar.activation(
                                ex[:],
                                ps_d[kh, loc][:],
                                mybir.ActivationFunctionType.Exp,
                                scale=0.125,
                            )
                            exps[kh, loc] = ex
                    ps_o = {
                        loc: psa.tile([65, 512], FP, tag="av", name=f"po{loc}")
                        for loc in locs
                    }
                    for kh in range(2):
                        for loc in locs:
                            nc.tensor.matmul(
                                ps_o[loc][:],
                                vpa[kh][:],
                                exps[kh, loc][:],
                                start=(kh == 0),
                                stop=(kh == 1),
                            )
                    for loc in locs:
                        s0 = half * 2048 + loc * 512
                        # rowsum row -> SBUF (single-lane, on ACT), broadcast
                        # to 64 partitions via rank-1 matmul, then full-width
                        # reciprocal and multiply on DVE
                        rs_sb = hot.tile([1, 512], FP, tag="rrow")
                        nc.scalar.copy(rs_sb[:], ps_o[loc][64:65, :])
                        bc = psb.tile([D, 512], FP, tag="bc")
                        nc.tensor.matmul(bc[:], ones_sb[:], rs_sb[:], start=True, stop=True)
                        rec_sb = hot.tile([D, 512], FP, tag="bcs")
                        nc.vector.reciprocal(rec_sb[:], bc[:])
                        nc.vector.tensor_mul(
                            osb[:, s0 : s0 + 512],
                            ps_o[loc][0:64, :],
                            rec_sb[:],
                        )
                nc.sync.dma_start(O[pr], osb[:])

    return nc


def _get_nc():
    global _nc_cache
    if _nc_cache is None:
        _nc_cache = build_nc()
        _nc_cache.compile()
    return _nc_cache


def _order_flats(flats):
    """Order so the duplicated head's two pairs come first."""
    hs = [f // 2 for f in flats]
    dup = next(h for h in hs if hs.count(h) == 2)
    return [f for f in flats if f // 2 == dup] + [f for f in flats if f // 2 != dup]


def _pack_s(x):
    """[S, F] -> [128, 32*F] with row p, col c*F+f = x[c*128+p, f]."""
    Sdim, F = x.shape
    return np.ascontiguousarray(
        x.reshape(SC, P, F).transpose(1, 0, 2).reshape(P, SC * F)
    )


def make_in_maps(Q, K, V, mask, E):
    Q = np.asarray(Q, np.float32)
    K = np.asarray(K, np.float32)
    V = np.asarray(V, np.float32)
    mask = np.asarray(mask, np.float32)
    E = np.asarray(E, np.float32)
    in_maps, metas = [], []
    for c in range(NC):
        flats = _order_flats([3 * c, 3 * c + 1, 3 * c + 2])
        pairs = [divmod(f, 2) for f in flats]  # (h, b)
        heads = [flats[0] // 2, flats[2] // 2]
        ET = np.stack([_pack_s(np.ascontiguousarray(E[h].T)) for h in heads])
        KVm = np.stack(
            [
                _pack_s(
                    np.concatenate([K[b, h], V[b, h]], axis=-1)
                    * mask[b][:, None]
                )
                for h, b in pairs
            ]
        )
        QTP = np.stack(
            [
                np.concatenate([Q[b, h].T[:, :2048], Q[b, h].T[:, 2048:]], axis=0)
                for h, b in pairs
            ]
        )
        in_maps.append(
            {
                "ET": np.ascontiguousarray(ET),
                "KV": np.ascontiguousarray(KVm),
                "QTP": np.ascontiguousarray(QTP),
            }
        )
        metas.append(pairs)
    return in_maps, metas


def unshard(results, metas):
    out = np.empty((B, H, S, D), np.float32)
    for c in range(NC):
        for i, (h, b) in enumerate(metas[c]):
            out[b, h] = results[c]["O"][i].T
    return out


def kernel(Q, K, V, mask, E, **run_kwargs):
    nc = _get_nc()
    in_maps, metas = make_in_maps(Q, K, V, mask, E)
    res = run_bass_kernel_spmd(nc, in_maps, core_ids=list(range(NC)), **run_kwargs)
    out = unshard(res.results, metas)
    kernel.last_result = res
    return out


# revision 13
# speedup vs baseline: 1.7096x; 1.6775x over previous
"""Linformer attention Bass/Trainium2 kernel (8 NeuronCores, SPMD).

Math per (b, h):
  Kp = E[h] @ (K[b,h] * m[b])            [LK, D]     (contraction over S)
  Vp = E[h] @ (V[b,h] * m[b])            [LK, D]
  dot = Q[b,h] @ Kp.T / sqrt(D)          [S, LK]
  attn = softmax(dot, axis=-1)
  out = attn @ Vp                        [S, D]

Shapes: B=2, H=12, S=4096, D=64, LK=256.

Sharding: 24 (h,b) pairs -> 8 cores x 3 pairs, h-major so each core touches
exactly 2 distinct heads (pairs ordered [dupA, dupA, single] so the SPMD
program reuses the first E head for pairs 0,1 and loads the second for pair 2).

Device-side layouts (host prepacks; s is tiled s = c*128 + p):
  ET  [2, 128, 32*256] : ET[i][p, c*256+k]  = E[head_i, k, c*128+p]   (E^T)
  KV  [3, 128, 32*128] : KV[pr][p, c*128+j] = concat(K,V)[b,h,c*128+p,j]
  QTP [3, 128, 4*512]  : QTP[pr][64*half+d, s'] = Q[b,h, 2048*half+s', d]
  MK  [3, 128, 32]     : MK[pr][p, c]       = mask[b, c*128+p]
  O   [3, 128, 32*64]  : O[pr][p, c*64+d]   = out[b,h, c*128+p, d]

On-device pipeline per pair:
  1. kv *= mask (per-partition broadcast)
  2. proj: psum[128,256] += kv_chunk[128s,128]^T @ et_chunk[128s,256]
     -> rows 0:64 = Kp^T[d,k], rows 64:128 = Vp^T[d,k]
  3. Vp^T -> PE-transpose -> Vp[k,d]; augment with ones column (rowsum trick)
  4. per s-chunk 512: dot^T[k_half,s] = Kp^T_half.T @ Q^T_chunk (2 matmuls),
     exp on ACT (scale=1/8, no max-subtraction: |dot/8| < ~10, fp32-safe),
     out_un^T[65,s] = [Vp;1]^T-augmented AV matmul over both k halves,
     PE-transpose 128-col pieces -> [128, 65], normalize by 1/rowsum
     (per-partition scalar) -> natural [s, d] layout, stage to osb.
  5. one 1MB DMA of osb -> O[pr]
"""

import sys

if "/opt/trn_rl_repo" not in sys.path:
    sys.path.insert(0, "/opt/trn_rl_repo")

import numpy as np

import concourse.bass as bass
import concourse.bacc as bacc
import concourse.mybir as mybir
import concourse.tile as tile
from concourse.bass_utils import run_bass_kernel_spmd
from concourse.masks import make_identity

FP = mybir.dt.float32

B, H, S, D, LK = 2, 12, 4096, 64, 256
NC = 8
PAIRS = 3          # (b,h) pairs per core
P = 128            # partitions
SC = S // P        # 32 s-chunks of 128
NSC = S // 512     # 8 s-chunks of 512

_nc_cache = None


def build_nc():
    nc = bacc.Bacc(None, target_bir_lowering=False, debug=False)
    ET = nc.declare_dram_parameter("ET", [2, P, SC * LK], FP, isOutput=False)
    KV = nc.declare_dram_parameter("KV", [PAIRS, P, SC * P], FP, isOutput=False)
    QTP = nc.declare_dram_parameter("QTP", [PAIRS, P, 4 * 512], FP, isOutput=False)
    O = nc.declare_dram_parameter("O", [PAIRS, D, S], FP, isOutput=True)

    with tile.TileContext(nc) as tc:
        with (
            tc.tile_pool(name="etp", bufs=2) as etp,
            tc.tile_pool(name="big", bufs=2) as big,
            tc.tile_pool(name="small", bufs=2) as small,
            tc.tile_pool(name="hot", bufs=6) as hot,
            tc.tile_pool(name="outp", bufs=3) as outp,
            tc.tile_pool(name="singles", bufs=1) as singles,
            tc.tile_pool(name="psd", bufs=3, space="PSUM") as psd,
            tc.tile_pool(name="psa", bufs=3, space="PSUM") as psa,
            tc.tile_pool(name="psb", bufs=1, space="PSUM") as psb,
        ):
            ident = singles.tile([P, P], FP)
            make_identity(nc, ident)

            et_tiles = [
                etp.tile([P, SC * LK], FP, tag="et", name=f"et{i}") for i in range(2)
            ]
            nc.sync.dma_start(et_tiles[0][:], ET[0])

            for pr in range(PAIRS):
                if pr == 1:
                    nc.sync.dma_start(et_tiles[1][:], ET[1])
                et_sb = et_tiles[0 if pr < 2 else 1]

                kv_sb = big.tile([P, SC * P], FP, tag="kv")
                nc.sync.dma_start(kv_sb[:], KV[pr])
                qtp_sb = big.tile([P, 4 * 512], FP, tag="qtp")
                nc.sync.dma_start(qtp_sb[:], QTP[pr])

                # projection: accumulate over all 32 s-chunks
                ps_p = psa.tile([P, LK], FP, tag="av")
                for c in range(SC):
                    nc.tensor.matmul(
                        ps_p[:],
                        kv_sb[:, c * P : (c + 1) * P],
                        et_sb[:, c * LK : (c + 1) * LK],
                        start=(c == 0),
                        stop=(c == SC - 1),
                    )

                # proj_sb rows 0:64 = Kp^T, rows 64:128 = Vp^T
                proj_sb = small.tile([P, LK], FP, tag="proj")
                nc.vector.tensor_copy(proj_sb[:], ps_p[:])
                # duplicate Kp^T into partitions 64:128 for the high s-half matmuls
                kpt_hi = small.tile([P, LK], FP, tag="kpt_hi")
                nc.sync.dma_start(kpt_hi[64:128, :], proj_sb[0:64, :])

                # Vp^T [64,256] -> Vp [256,64] as two [128,65] lhsT tiles
                # (column 64 = ones row for the softmax denominator)
                vpa = []
                for kh in range(2):
                    ps_t = psb.tile([P, 64], FP, tag="bc")
                    nc.tensor.transpose(
                        ps_t[:],
                        proj_sb[64:128, kh * P : (kh + 1) * P],
                        ident[64:128, 64:128],
                    )
                    va = small.tile([P, P], FP, tag=f"vpa{kh}")
                    nc.vector.memset(va[:, 0:64], 1.0)
                    nc.vector.tensor_copy(va[:, 64:128], ps_t[:])
                    vpa.append(va)

                osb = outp.tile([P, S], FP, tag="osb")

                # 8 s-chunks of 512, in groups of 2 sharing dot weights;
                # groups alternate low/high s-halves so consecutive dot
                # matmuls use disjoint PE row groups (partitions 0:64/64:128)
                for half, locs in ((0, (0, 1)), (1, (0, 1)), (0, (2, 3)), (1, (2, 3))):
                    pb = 64 * half
                    lhs_src = kpt_hi if half else proj_sb
                    ps_d = {}
                    for kh in range(2):
                        for loc in locs:
                            pd = psd.tile([P, 512], FP, tag="dot", name=f"pd{kh}{loc}")
                            nc.tensor.matmul(
                                pd[:],
                                lhs_src[pb : pb + 64, kh * P : (kh + 1) * P],
                                qtp_sb[pb : pb + 64, loc * 512 : (loc + 1) * 512],
                                start=True,
                                stop=True,
                            )
                            ps_d[kh, loc] = pd
                    exps = {}
                    for kh in range(2):
                        for loc in locs:
                            ex = hot.tile([P, 512], FP, tag="exp", name=f"ex{kh}{loc}")
                            nc.scalar.activation(
                                ex[:],
                                ps_d[kh, loc][:],
                                mybir.ActivationFunctionType.Exp,
                                scale=0.125,
                            )
                            exps[kh, loc] = ex
                    ps_o = {
                        loc: psa.tile([P, 512], FP, tag="av", name=f"po{loc}")
                        for loc in locs
                    }
                    for kh in range(2):
                        for loc in locs:
                            nc.tensor.matmul(
                                ps_o[loc][:],
                                vpa[kh][:],
                                exps[kh, loc][:],
                                start=(kh == 0),
                                stop=(kh == 1),
                            )
                    for loc in locs:
                        s0 = half * 2048 + loc * 512
                        # ps_o rows 0:64 = rowsum (ones-block), 64:128 = values.
                        # recip at partitions 0:64, DMA-shift it to 64:128
                        # (lane-aligned with the values), multiply there.
                        scr = hot.tile([P, 512], FP, tag="scr")
                        rec = hot.tile([P, 512], FP, tag="bcs")
                        nc.vector.reciprocal_approx_accurate(
                            rec[0:64, :], ps_o[loc][0:64, :], scr[0:64, :]
                        )
                        nc.sync.dma_start(rec[64:128, :], rec[0:64, :])
                        nc.vector.tensor_mul(
                            osb[64:128, s0 : s0 + 512],
                            ps_o[loc][64:128, :],
                            rec[64:128, :],
                        )
                nc.sync.dma_start(O[pr], osb[64:128, :])

    return nc


def _get_nc():
    global _nc_cache
    if _nc_cache is None:
        _nc_cache = build_nc()
        _nc_cache.compile()
    return _nc_cache


def _order_flats(flats):
    """Order so the duplicated head's two pairs come first."""
    hs = [f // 2 for f in flats]
    dup = next(h for h in hs if hs.count(h) == 2)
    return [f for f in flats if f // 2 == dup] + [f for f in flats if f // 2 != dup]


def _pack_s(x):
    """[S, F] -> [128, 32*F] with row p, col c*F+f = x[c*128+p, f]."""
    Sdim, F = x.shape
    return np.ascontiguousarray(
        x.reshape(SC, P, F).transpose(1, 0, 2).reshape(P, SC * F)
    )


def make_in_maps(Q, K, V, mask, E):
    Q = np.asarray(Q, np.float32)
    K = np.asarray(K, np.float32)
    V = np.asarray(V, np.float32)
    mask = np.asarray(mask, np.float32)
    E = np.asarray(E, np.float32)
    in_maps, metas = [], []
    for c in range(NC):
        flats = _order_flats([3 * c, 3 * c + 1, 3 * c + 2])
        pairs = [divmod(f, 2) for f in flats]  # (h, b)
        heads = [flats[0] // 2, flats[2] // 2]
        ET = np.stack([_pack_s(np.ascontiguousarray(E[h].T)) for h in heads])
        KVm = np.stack(
            [
                _pack_s(
                    np.concatenate([K[b, h], V[b, h]], axis=-1)
                    * mask[b][:, None]
                )
                for h, b in pairs
            ]
        )
        QTP = np.stack(
            [
                np.concatenate([Q[b, h].T[:, :2048], Q[b, h].T[:, 2048:]], axis=0)
                for h, b in pairs
            ]
        )
        in_maps.append(
            {
                "ET": np.ascontiguousarray(ET),
                "KV": np.ascontiguousarray(KVm),
                "QTP": np.ascontiguousarray(QTP),
            }
        )
        metas.append(pairs)
    return in_maps, metas


def unshard(results, metas):
    out = np.empty((B, H, S, D), np.float32)
    for c in range(NC):
        for i, (h, b) in enumerate(metas[c]):
            out[b, h] = results[c]["O"][i].T
    return out


def kernel(Q, K, V, mask, E, **run_kwargs):
    nc = _get_nc()
    in_maps, metas = make_in_maps(Q, K, V, mask, E)
    res = run_bass_kernel_spmd(nc, in_maps, core_ids=list(range(NC)), **run_kwargs)
    out = unshard(res.results, metas)
    kernel.last_result = res
    return out


# revision 14
# speedup vs baseline: 1.7515x; 1.0245x over previous
"""Linformer attention Bass/Trainium2 kernel (8 NeuronCores, SPMD).

Math per (b, h):
  Kp = E[h] @ (K[b,h] * m[b])            [LK, D]     (contraction over S)
  Vp = E[h] @ (V[b,h] * m[b])            [LK, D]
  dot = Q[b,h] @ Kp.T / sqrt(D)          [S, LK]
  attn = softmax(dot, axis=-1)
  out = attn @ Vp                        [S, D]

Shapes: B=2, H=12, S=4096, D=64, LK=256.

Sharding: 24 (h,b) pairs -> 8 cores x 3 pairs, h-major so each core touches
exactly 2 distinct heads (pairs ordered [dupA, dupA, single] so the SPMD
program reuses the first E head for pairs 0,1 and loads the second for pair 2).

Device-side layouts (host prepacks; s is tiled s = c*128 + p):
  ET  [2, 128, 32*256] : ET[i][p, c*256+k]  = E[head_i, k, c*128+p]   (E^T)
  KV  [3, 128, 32*128] : KV[pr][p, c*128+j] = concat(K,V)[b,h,c*128+p,j]
  QTP [3, 128, 4*512]  : QTP[pr][64*half+d, s'] = Q[b,h, 2048*half+s', d]
  MK  [3, 128, 32]     : MK[pr][p, c]       = mask[b, c*128+p]
  O   [3, 128, 32*64]  : O[pr][p, c*64+d]   = out[b,h, c*128+p, d]

On-device pipeline per pair:
  1. kv *= mask (per-partition broadcast)
  2. proj: psum[128,256] += kv_chunk[128s,128]^T @ et_chunk[128s,256]
     -> rows 0:64 = Kp^T[d,k], rows 64:128 = Vp^T[d,k]
  3. Vp^T -> PE-transpose -> Vp[k,d]; augment with ones column (rowsum trick)
  4. per s-chunk 512: dot^T[k_half,s] = Kp^T_half.T @ Q^T_chunk (2 matmuls),
     exp on ACT (scale=1/8, no max-subtraction: |dot/8| < ~10, fp32-safe),
     out_un^T[65,s] = [Vp;1]^T-augmented AV matmul over both k halves,
     PE-transpose 128-col pieces -> [128, 65], normalize by 1/rowsum
     (per-partition scalar) -> natural [s, d] layout, stage to osb.
  5. one 1MB DMA of osb -> O[pr]
"""

import sys

if "/opt/trn_rl_repo" not in sys.path:
    sys.path.insert(0, "/opt/trn_rl_repo")

import numpy as np

import concourse.bass as bass
import concourse.bacc as bacc
import concourse.mybir as mybir
import concourse.tile as tile
from concourse.bass_utils import run_bass_kernel_spmd
from concourse.masks import make_identity

FP = mybir.dt.float32

B, H, S, D, LK = 2, 12, 4096, 64, 256
NC = 8
PAIRS = 3          # (b,h) pairs per core
P = 128            # partitions
SC = S // P        # 32 s-chunks of 128
NSC = S // 512     # 8 s-chunks of 512

_nc_cache = None


def build_nc():
    nc = bacc.Bacc(None, target_bir_lowering=False, debug=False)
    ET = nc.declare_dram_parameter("ET", [2, P, SC * LK], FP, isOutput=False)
    KV = nc.declare_dram_parameter("KV", [PAIRS, P, SC * P], FP, isOutput=False)
    QTP = nc.declare_dram_parameter("QTP", [PAIRS, P, 4 * 512], FP, isOutput=False)
    O = nc.declare_dram_parameter("O", [PAIRS, D, S], FP, isOutput=True)

    with tile.TileContext(nc) as tc:
        with (
            tc.tile_pool(name="etp", bufs=2) as etp,
            tc.tile_pool(name="big", bufs=2) as big,
            tc.tile_pool(name="small", bufs=2) as small,
            tc.tile_pool(name="hot", bufs=6) as hot,
            tc.tile_pool(name="outp", bufs=3) as outp,
            tc.tile_pool(name="singles", bufs=1) as singles,
            tc.tile_pool(name="psd", bufs=4, space="PSUM") as psd,
            tc.tile_pool(name="psa", bufs=3, space="PSUM") as psa,
            tc.tile_pool(name="psb", bufs=1, space="PSUM") as psb,
        ):
            ident = singles.tile([P, P], FP)
            make_identity(nc, ident)

            et_tiles = [
                etp.tile([P, SC * LK], FP, tag="et", name=f"et{i}") for i in range(2)
            ]
            nc.sync.dma_start(et_tiles[0][:], ET[0])

            for pr in range(PAIRS):
                if pr == 1:
                    nc.sync.dma_start(et_tiles[1][:], ET[1])
                et_sb = et_tiles[0 if pr < 2 else 1]

                kv_sb = big.tile([P, SC * P], FP, tag="kv")
                nc.sync.dma_start(kv_sb[:], KV[pr])
                qtp_sb = big.tile([P, 4 * 512], FP, tag="qtp")
                nc.sync.dma_start(qtp_sb[:], QTP[pr])

                # projection: accumulate over all 32 s-chunks
                ps_p = psa.tile([P, LK], FP, tag="av")
                for c in range(SC):
                    nc.tensor.matmul(
                        ps_p[:],
                        kv_sb[:, c * P : (c + 1) * P],
                        et_sb[:, c * LK : (c + 1) * LK],
                        start=(c == 0),
                        stop=(c == SC - 1),
                    )

                # proj_sb rows 0:64 = Kp^T, rows 64:128 = Vp^T
                proj_sb = small.tile([P, LK], FP, tag="proj")
                nc.vector.tensor_copy(proj_sb[:], ps_p[:])
                # duplicate Kp^T into partitions 64:128 for the high s-half matmuls
                kpt_hi = small.tile([P, LK], FP, tag="kpt_hi")
                nc.sync.dma_start(kpt_hi[64:128, :], proj_sb[0:64, :])

                # Vp^T [64,256] -> Vp [256,64] as two [128,65] lhsT tiles
                # (column 64 = ones row for the softmax denominator)
                vpa = []
                for kh in range(2):
                    ps_t = psb.tile([P, 64], FP, tag="bc")
                    nc.tensor.transpose(
                        ps_t[:],
                        proj_sb[64:128, kh * P : (kh + 1) * P],
                        ident[64:128, 64:128],
                    )
                    va = small.tile([P, P], FP, tag=f"vpa{kh}")
                    nc.vector.memset(va[:, 0:64], 1.0)
                    nc.vector.tensor_copy(va[:, 64:128], ps_t[:])
                    vpa.append(va)

                osb = outp.tile([P, S], FP, tag="osb")

                # 4 groups; each group handles one 512-wide s-chunk from BOTH
                # s-halves. Low-half dot matmuls run on PE rows 0:64, high-half
                # on rows 64:128 (disjoint row groups -> concurrent execution).
                for pg in range(4):
                    ps_d = {}
                    for kh in range(2):
                        for half in (0, 1):
                            pb = 64 * half
                            lhs_src = kpt_hi if half else proj_sb
                            pd = psd.tile([P, 512], FP, tag="dot", name=f"pd{kh}{half}")
                            nc.tensor.matmul(
                                pd[:],
                                lhs_src[pb : pb + 64, kh * P : (kh + 1) * P],
                                qtp_sb[pb : pb + 64, pg * 512 : (pg + 1) * 512],
                                start=True,
                                stop=True,
                            )
                            ps_d[kh, half] = pd
                    exps = {}
                    for kh in range(2):
                        for half in (0, 1):
                            ex = hot.tile([P, 512], FP, tag="exp", name=f"ex{kh}{half}")
                            nc.scalar.activation(
                                ex[:],
                                ps_d[kh, half][:],
                                mybir.ActivationFunctionType.Exp,
                                scale=0.125,
                            )
                            exps[kh, half] = ex
                    ps_o = {
                        half: psa.tile([P, 512], FP, tag="av", name=f"po{half}")
                        for half in (0, 1)
                    }
                    for kh in range(2):
                        for half in (0, 1):
                            nc.tensor.matmul(
                                ps_o[half][:],
                                vpa[kh][:],
                                exps[kh, half][:],
                                start=(kh == 0),
                                stop=(kh == 1),
                            )
                    for half in (0, 1):
                        s0 = half * 2048 + pg * 512
                        # ps_o rows 0:64 = rowsum (ones-block), 64:128 = values.
                        # recip at partitions 0:64, DMA-shift it to 64:128
                        # (lane-aligned with the values), multiply there.
                        scr = hot.tile([P, 512], FP, tag="scr")
                        rec = hot.tile([P, 512], FP, tag="bcs")
                        nc.vector.reciprocal_approx_accurate(
                            rec[0:64, :], ps_o[half][0:64, :], scr[0:64, :]
                        )
                        nc.sync.dma_start(rec[64:128, :], rec[0:64, :])
                        nc.vector.tensor_mul(
                            osb[64:128, s0 : s0 + 512],
                            ps_o[half][64:128, :],
                            rec[64:128, :],
                        )
                nc.sync.dma_start(O[pr], osb[64:128, :])

    return nc


def _get_nc():
    global _nc_cache
    if _nc_cache is None:
        _nc_cache = build_nc()
        _nc_cache.compile()
    return _nc_cache


def _order_flats(flats):
    """Order so the duplicated head's two pairs come first."""
    hs = [f // 2 for f in flats]
    dup = next(h for h in hs if hs.count(h) == 2)
    return [f for f in flats if f // 2 == dup] + [f for f in flats if f // 2 != dup]


def _pack_s(x):
    """[S, F] -> [128, 32*F] with row p, col c*F+f = x[c*128+p, f]."""
    Sdim, F = x.shape
    return np.ascontiguousarray(
        x.reshape(SC, P, F).transpose(1, 0, 2).reshape(P, SC * F)
    )


def make_in_maps(Q, K, V, mask, E):
    Q = np.asarray(Q, np.float32)
    K = np.asarray(K, np.float32)
    V = np.asarray(V, np.float32)
    mask = np.asarray(mask, np.float32)
    E = np.asarray(E, np.float32)
    in_maps, metas = [], []
    for c in range(NC):
        flats = _order_flats([3 * c, 3 * c + 1, 3 * c + 2])
        pairs = [divmod(f, 2) for f in flats]  # (h, b)
        heads = [flats[0] // 2, flats[2] // 2]
        ET = np.stack([_pack_s(np.ascontiguousarray(E[h].T)) for h in heads])
        KVm = np.stack(
            [
                _pack_s(
                    np.concatenate([K[b, h], V[b, h]], axis=-1)
                    * mask[b][:, None]
                )
                for h, b in pairs
            ]
        )
        QTP = np.stack(
            [
                np.concatenate([Q[b, h].T[:, :2048], Q[b, h].T[:, 2048:]], axis=0)
                for h, b in pairs
            ]
        )
        in_maps.append(
            {
                "ET": np.ascontiguousarray(ET),
                "KV": np.ascontiguousarray(KVm),
                "QTP": np.ascontiguousarray(QTP),
            }
        )
        metas.append(pairs)
    return in_maps, metas


def unshard(results, metas):
    out = np.empty((B, H, S, D), np.float32)
    for c in range(NC):
        for i, (h, b) in enumerate(metas[c]):
            out[b, h] = results[c]["O"][i].T
    return out


def kernel(Q, K, V, mask, E, **run_kwargs):
    nc = _get_nc()
    in_maps, metas = make_in_maps(Q, K, V, mask, E)
    res = run_bass_kernel_spmd(nc, in_maps, core_ids=list(range(NC)), **run_kwargs)
    out = unshard(res.results, metas)
    kernel.last_result = res
    return out
